# revision 3
# baseline (speedup 1.0000x reference)
"""Trainium2 Bass kernel for residual-VQ autoencoder (nn_Autoencoder_45148696216751).

Pipeline per core (data-parallel over tokens, 8 cores x 2048 tokens, no collectives):
  encoder zT = enc_w @ x.T (+bias)  -> residual rT [d, t] in SBUF
  2x VQ stage:
     score[t,k] = r.c - |c|^2/2 via PE matmuls, codebook streamed from HBM
                  (-|c|^2/2 folded into the stream as a 5th K=128 row)
     mode fp32:  exact fp32 matmuls; per-superchunk max8 + stt index extraction
     mode f32r4: full-speed f32r matmuls; per-superchunk top-8 (max8+max_index),
                 global top-4 by value, exact rescore of the 4 candidates
                 against the exact residual (gathered code rows, gpsimd math)
     indirect-DMA gather of the winning code row per token
     PE transpose of gathered rows; residual update / q_sum accumulation
  decoder out = q_sum @ dec_w.T + dec_b -> DMA out
"""
import sys, types, os

sys.path.insert(0, '/opt/trn_rl_repo')
import numpy as np

import concourse.bass as bass
import concourse.tile as tile
from concourse import bacc, mybir
from concourse.bass_utils import run_bass_kernel_spmd
from concourse.masks import make_identity

f32 = mybir.dt.float32
f32r = mybir.dt.float32r
i32 = mybir.dt.int32
u32 = mybir.dt.uint32
ALU = mybir.AluOpType

NCORES = 8
B, N, D = 4, 4096, 512
T = B * N                 # 16384 tokens
TL = T // NCORES          # 2048 tokens per core
K = 16384                 # codebook size
NT = TL // 128            # 16 token tiles per core
NJ = D // 128             # 4 contraction tiles
SC = 1024                 # superchunk (2 psum banks)
NSC = K // SC             # 16 superchunks
NR = 5                    # codebook stream rows (4 cb + 1 bias)
DX = 520                  # rescore row: 512 code dims + csq + pad
NUM_Q = 2
NCAND = 3                 # rescued candidates per token

DIST_MODE = os.environ.get("VQ_DIST_MODE", "bf16r4")  # fp32 | f32r4 | bf16r4


def _ensure_axon_hook():
    """Register the NTFF profile hook (missing antenv.axon_hooks shim)."""
    if "antenv.axon_hooks" in sys.modules:
        return
    mod = types.ModuleType("antenv.axon_hooks")
    _h = [None]
    mod.set_axon_ntff_profile_hook = lambda h: _h.__setitem__(0, h)
    mod.get_axon_ntff_profile_hook = lambda: _h[0]
    sys.modules["antenv.axon_hooks"] = mod
    try:
        import antenv
        antenv.axon_hooks = mod
        from trn_agent_boot.trn_boot import _ntff_profile_via_ctypes
        hook = _ntff_profile_via_ctypes('/opt/axon/libaxon_pjrt.so')
        if hook is not None:
            mod.set_axon_ntff_profile_hook(hook)
    except Exception:
        pass


def _build(dist_mode):
    nc = bacc.Bacc("TRN2", target_bir_lowering=False, debug=False,
                   num_devices=NCORES)
    rescue = dist_mode in ("f32r4", "bf16r4")
    md = {"fp32": f32, "f32r4": f32r, "bf16r4": mybir.dt.bfloat16}[dist_mode]

    xT_d = nc.dram_tensor("xT", [128, NJ, TL], f32, kind="ExternalInput")
    cbs_d = nc.dram_tensor("cbs", [128, NR, K], md, kind="ExternalInput")
    cb_d = nc.dram_tensor("cb", [K, D], f32, kind="ExternalInput")
    cbx_d = nc.dram_tensor("cbx", [K, DX], f32, kind="ExternalInput")
    ewT_d = nc.dram_tensor("ewT", [128, NJ, D], f32, kind="ExternalInput")
    dwT_d = nc.dram_tensor("dwT", [128, NJ, D], mybir.dt.bfloat16,
                           kind="ExternalInput")
    eb_d = nc.dram_tensor("eb", [128, NJ], f32, kind="ExternalInput")
    db_d = nc.dram_tensor("db", [128, D], f32, kind="ExternalInput")
    ones_d = nc.dram_tensor("ones", [128, 128], md, kind="ExternalInput")
    if rescue:
        ebf_d = nc.dram_tensor("ebf", [128, D], f32, kind="ExternalInput")
    out_d = nc.dram_tensor("out", [TL, D], f32, kind="ExternalOutput")

    from contextlib import ExitStack
    with tile.TileContext(nc) as tc, ExitStack() as ctx:
        big = ctx.enter_context(tc.tile_pool(name="big", bufs=1))
        scrp = ctx.enter_context(tc.tile_pool(name="scr", bufs=2))
        smallp = ctx.enter_context(tc.tile_pool(name="small", bufs=8))
        qp = ctx.enter_context(tc.tile_pool(name="qp", bufs=3))
        q4p = ctx.enter_context(tc.tile_pool(name="q4p", bufs=3))
        qbp = ctx.enter_context(tc.tile_pool(name="qbp", bufs=3))
        qtp = ctx.enter_context(tc.tile_pool(name="qtp", bufs=3))
        outp = ctx.enter_context(tc.tile_pool(name="outp", bufs=2))
        psc = ctx.enter_context(tc.tile_pool(name="psc", bufs=3, space="PSUM"))
        psm = ctx.enter_context(tc.tile_pool(name="psm", bufs=2, space="PSUM"))

        bf16 = mybir.dt.bfloat16
        # ---- persistent tiles
        rT = big.tile([128, NJ, TL], md)      # residual (transposed)
        q1T = big.tile([128, NJ, TL], bf16)   # q1T, later q_sumT
        ewT = big.tile([128, NJ, D], f32)
        dwT = big.tile([128, NJ, D], bf16)
        db = big.tile([128, D], f32)
        ones128 = big.tile([128, 128], md)
        m8buf = big.tile([128, NT, NSC, 8], f32)
        if rescue:
            z_td = big.tile([128, NT, DX], f32)   # exact residual + csq slot
            ebf = big.tile([128, D], f32)
            idxbuf = big.tile([128, NT, NSC, 8], mybir.dt.uint16)
            iota_off8 = big.tile([128, NSC, 8], f32)
            wi32 = big.tile([128, NT], i32)
        else:
            iota_sc = big.tile([128, SC], f32)
            iota_off = big.tile([128, NSC], f32)
            sidxbuf = big.tile([128, NT, NSC], f32)
            idx32 = big.tile([128, NT], i32)

        nc.sync.dma_start(ewT[:], ewT_d.ap())
        nc.sync.dma_start(dwT[:], dwT_d.ap())
        nc.sync.dma_start(db[:], db_d.ap())
        nc.sync.dma_start(ones128[:], ones_d.ap())
        if rescue:
            nc.sync.dma_start(ebf[:], ebf_d.ap())
        if rescue:
            zpad = z_td[:, :, D:DX].bitcast(u32)
            nc.vector.tensor_scalar(out=zpad, in0=zpad, scalar1=0,
                                    scalar2=None, op0=ALU.mult)
            nc.gpsimd.iota(iota_off8[:], pattern=[[SC, NSC], [0, 8]], base=0,
                           channel_multiplier=0,
                           allow_small_or_imprecise_dtypes=True)
        else:
            nc.gpsimd.iota(iota_sc[:], pattern=[[1, SC]], base=0,
                           channel_multiplier=0,
                           allow_small_or_imprecise_dtypes=True)
            nc.gpsimd.iota(iota_off[:], pattern=[[SC, NSC]], base=0,
                           channel_multiplier=0,
                           allow_small_or_imprecise_dtypes=True)

        # ---- encoder: z_td (exact fp32, [t,d]); rT = bf16 transpose of z.
        # Tiles 0-7 inline (feed first sweep); tiles 8-15 deferred into sweep A.
        xp = ctx.enter_context(tc.tile_pool(name="xp", bufs=3))
        zbp = ctx.enter_context(tc.tile_pool(name="zbp", bufs=4))
        enc_steps = []

        def enc_tile(t):
            xtt = xp.tile([128, NJ, 128], f32, tag="x")
            nc.sync.dma_start(xtt[:], xT_d.ap()[:, :, t * 128:(t + 1) * 128])
            psz = psm.tile([128, 512], f32, tag="m")
            for j in range(NJ):
                nc.tensor.matmul(
                    psz[:], lhsT=xtt[:, j, :],
                    rhs=ewT[:, j, :],
                    start=(j == 0), stop=(j == NJ - 1))
            nc.vector.tensor_add(z_td[:, t, :D], psz[:], ebf[:])
            ztb = zbp.tile([128, D], bf16, tag="zb")
            nc.vector.tensor_copy(ztb[:], z_td[:, t, :D])
            for j in range(NJ):
                nc.scalar.dma_start_transpose(
                    rT[:, j, t * 128:(t + 1) * 128],
                    ztb[:, j * 128:(j + 1) * 128])

        for t in range(8):
            enc_tile(t)
        for t in range(8, 16):
            enc_steps.append(lambda t=t: enc_tile(t))

        with tc.tile_pool(name="cbp", bufs=6) as cbp:
            # ---- VQ stages, combine steps interleaved into the next sweep
            assert rescue
            pending = []

            def drain(n):
                for _ in range(min(n, len(pending))):
                    pending.pop(0)()

            def pre1(t):
                st = {}
                idxf = smallp.tile([128, NSC, 8], f32, tag="idxf")
                nc.vector.tensor_copy(idxf[:], idxbuf[:, t, :, :])
                nc.vector.tensor_add(idxf[:], idxf[:], iota_off8[:])
                g8 = smallp.tile([128, 8], f32, tag="g8")
                nc.vector.max(out=g8[:], in_=m8buf[:, t, :, :])
                cs = smallp.tile([128, NCAND], f32, tag="cs")
                junk = scrp.tile([128, NSC, 8], f32, tag="s")
                for k_ in range(NCAND):
                    nc.vector.scalar_tensor_tensor(
                        out=junk[:], in0=m8buf[:, t, :, :],
                        scalar=g8[:, k_:k_ + 1], in1=idxf[:],
                        op0=ALU.is_ge, op1=ALU.mult,
                        accum_out=cs[:, k_:k_ + 1])
                idx4f = smallp.tile([128, NCAND], f32, tag="i4f")
                nc.vector.tensor_copy(idx4f[:, 0:1], cs[:, 0:1])
                nc.vector.tensor_sub(idx4f[:, 1:NCAND], cs[:, 1:NCAND],
                                     cs[:, 0:NCAND - 1])
                idx4 = smallp.tile([128, NCAND], i32, tag="i4")
                nc.vector.tensor_copy(idx4[:], idx4f[:])
                st["idx4f"] = idx4f
                st["idx4"] = idx4
                return st

            def pre2(t, st):
                idx4 = st["idx4"]
                # u4 = z (+ gathered code row, csq in slot 512) per cand
                u4 = q4p.tile([128, NCAND, DX], f32, tag="u4")
                for k_ in range(NCAND):
                    nc.scalar.copy(u4[:, k_, :], z_td[:, t, :])
                for k_ in range(NCAND):
                    nc.gpsimd.indirect_dma_start(
                        out=u4[:, k_, :], out_offset=None,
                        in_=cbx_d.ap(),
                        in_offset=bass.IndirectOffsetOnAxis(
                            ap=idx4[:, k_:k_ + 1], axis=0),
                        compute_op=ALU.add)
                st["u4"] = u4

            def post_a(t, st, s):
                u4 = st["u4"]
                idx4f = st["idx4f"]
                # rank by sum((q+z)^2) - 2*csq  (== 2*(q.z - csq/2) + const)
                s1 = smallp.tile([128, NCAND], f32, tag="s1")
                junku = scrp.tile([128, NCAND, D], f32, tag="u4j")
                for k_ in range(NCAND):
                    nc.scalar.activation(
                        out=junku[:, k_, :], in_=u4[:, k_, :D],
                        func=mybir.ActivationFunctionType.Square,
                        accum_out=s1[:, k_:k_ + 1])
                sc4 = smallp.tile([128, NCAND], f32, tag="sc4")
                nc.vector.scalar_tensor_tensor(
                    out=sc4[:], in0=u4[:, :, D], scalar=-2.0, in1=s1[:],
                    op0=ALU.mult, op1=ALU.add)
                gm = smallp.tile([128, 1], f32, tag="gm")
                nc.vector.tensor_reduce(gm[:], sc4[:],
                                        axis=mybir.AxisListType.X,
                                        op=ALU.max)
                junk4 = smallp.tile([128, NCAND], f32, tag="j4")
                widxf = smallp.tile([128, 1], f32, tag="wf")
                nc.vector.scalar_tensor_tensor(
                    out=junk4[:], in0=sc4[:], scalar=gm[:],
                    in1=idx4f[:], op0=ALU.is_ge, op1=ALU.mult,
                    accum_out=widxf[:])
                nc.vector.tensor_copy(wi32[:, t:t + 1], widxf[:])
                q_t = qp.tile([128, D], f32, tag="qt")
                nc.gpsimd.indirect_dma_start(
                    out=q_t[:], out_offset=None, in_=cb_d.ap(),
                    in_offset=bass.IndirectOffsetOnAxis(
                        ap=wi32[:, t:t + 1], axis=0))
                st["q_t"] = q_t

            def post_t(t, st, s):
                q_t = st["q_t"]
                if s == 0:
                    # exact residual for stage-2 rescoring
                    nc.vector.tensor_sub(z_td[:, t, :D], z_td[:, t, :D],
                                         q_t[:])
                # transpose off the PE: bf16 cast (ACT) + DMA xbar
                q_tb = qbp.tile([128, D], bf16, tag="qb")
                nc.scalar.copy(q_tb[:], q_t[:])
                qT = qtp.tile([128, NJ, 128], bf16, tag="qT")
                for j in range(NJ):
                    nc.scalar.dma_start_transpose(
                        qT[:, j, :], q_tb[:, j * 128:(j + 1) * 128])
                st["qT"] = qT

            def post_b(t, st, s):
                qT = st["qT"]
                tsl = slice(t * 128, (t + 1) * 128)
                if s == 0:
                    nc.vector.tensor_copy(q1T[:, :, tsl], qT[:])
                    nc.vector.tensor_sub(rT[:, :, tsl], rT[:, :, tsl],
                                         qT[:])
                else:
                    nc.vector.tensor_add(q1T[:, :, tsl],
                                         q1T[:, :, tsl], qT[:])

            def decode(t):
                pso = psm.tile([128, 512], f32, tag="m")
                for j in range(NJ):
                    nc.tensor.matmul(
                        pso[:], lhsT=q1T[:, j, t * 128:(t + 1) * 128],
                        rhs=dwT[:, j, :],
                        start=(j == 0), stop=(j == NJ - 1))
                o_t = outp.tile([128, D], f32, tag="o")
                nc.vector.tensor_add(o_t[:], pso[:], db[:])
                nc.sync.dma_start(out_d.ap()[t * 128:(t + 1) * 128, :],
                                  o_t[:])

            def queue_combines(tset, s):
                sts = {}
                tlist = list(tset)
                phases = [
                    lambda t: sts.__setitem__(t, pre1(t)),
                    lambda t: pre2(t, sts[t]),
                    lambda t: post_a(t, sts[t], s),
                    lambda t: post_t(t, sts[t], s),
                    lambda t: post_b(t, sts[t], s),
                ]
                lags = [0, 1, 2, 2, 2]
                if s == 1:
                    phases.append(lambda t: decode(t))
                    lags.append(3)
                nph = len(phases)
                n = len(tlist)
                for i_ in range(n + max(lags)):
                    for ph in range(nph):
                        j_ = i_ - lags[ph]
                        if 0 <= j_ < n:
                            pending.append(
                                lambda ph=ph, t=tlist[j_]: phases[ph](t))

            pending.extend(enc_steps)
            for s in range(NUM_Q):
              groups = [range(0, 8), range(8, 16)]
              for tset in groups:
                # score sweep, draining pending combines of the previous group
                for sc in range(NSC):
                    cbts = []
                    for h in range(SC // 512):
                        cbt = cbp.tile([128, NR, 512], md, tag="cbt")
                        ko = sc * SC + h * 512
                        nc.sync.dma_start(cbt[:],
                                          cbs_d.ap()[:, :, ko:ko + 512])
                        cbts.append(cbt)
                    for t in tset:
                        ps = psc.tile([128, SC], f32, tag="sc")
                        for h in range(SC // 512):
                            pslice = ps[:, h * 512:(h + 1) * 512]
                            cbt = cbts[h]
                            for j in range(NJ):
                                nc.tensor.matmul(
                                    pslice,
                                    lhsT=rT[:, j, t * 128:(t + 1) * 128],
                                    rhs=cbt[:, j, :],
                                    start=(j == 0), stop=False)
                            nc.tensor.matmul(
                                pslice, lhsT=ones128[:],
                                rhs=cbt[:, NR - 1, :],
                                start=False, stop=True)
                        nc.vector.max(out=m8buf[:, t, sc, :], in_=ps[:])
                        nc.vector.max_index(out=idxbuf[:, t, sc, :],
                                            in_max=m8buf[:, t, sc, :],
                                            in_values=ps[:])
                    drain(3 if len(pending) > 2 * (NSC - 1 - sc) else 2)
                queue_combines(tset, s)

        # decode happens as combine phase 6; flush the remaining steps
        drain(len(pending))

    nc.compile()
    return nc


_CACHE = {}


def _get_nc():
    key = DIST_MODE
    if key not in _CACHE:
        _ensure_axon_hook()
        _CACHE[key] = _build(DIST_MODE)
    return _CACHE[key]


def _trunc_mant(a, bits):
    """Truncate fp32 mantissa to `bits` explicit bits (round to nearest)."""
    u = a.astype(np.float32).view(np.uint32).copy()
    shift = np.uint32(23 - bits)
    rb = np.uint32(1 << (23 - bits - 1))
    u = ((u + rb) >> shift) << shift
    return u.view(np.float32)


def _host_prep(x, enc_w, enc_b, codebook, dec_w, dec_b):
    x = np.asarray(x, np.float32)
    enc_w = np.asarray(enc_w, np.float32)
    enc_b = np.asarray(enc_b, np.float32)
    cb = np.ascontiguousarray(np.asarray(codebook, np.float32))
    dec_w = np.asarray(dec_w, np.float32)
    dec_b = np.asarray(dec_b, np.float32)
    rescue = DIST_MODE in ("f32r4", "bf16r4")

    flat = x.reshape(T, D)
    csq = (cb.astype(np.float64) ** 2).sum(-1).astype(np.float32)
    bias = -0.5 * csq
    cbx = np.concatenate(
        [cb, csq[:, None], np.zeros((K, DX - D - 1), np.float32)],
        axis=1).astype(np.float32)

    cbT = np.ascontiguousarray(cb.T)                      # [D, K]
    cbs = np.zeros((128, NR, K), np.float32)
    cbs[:, :NJ, :] = cbT.reshape(NJ, 128, K).transpose(1, 0, 2)
    if DIST_MODE == "f32r4":
        # split the bias exactly across two f32r-representable rows
        bh = _trunc_mant(bias, 10)
        cbs[0, NJ, :] = bh
        cbs[1, NJ, :] = bias - bh
    elif DIST_MODE == "bf16r4":
        import ml_dtypes
        bh = np.asarray(bias, ml_dtypes.bfloat16).astype(np.float32)
        cbs[0, NJ, :] = bh
        cbs[1, NJ, :] = bias - bh
        cbs = np.asarray(cbs, ml_dtypes.bfloat16)
    else:
        cbs[0, NJ, :] = bias

    ewT = np.ascontiguousarray(
        enc_w.T.reshape(NJ, 128, D).transpose(1, 0, 2))   # [128, NJ, D]
    import ml_dtypes
    dwT = np.ascontiguousarray(
        dec_w.T.reshape(NJ, 128, D).transpose(1, 0, 2)).astype(
            ml_dtypes.bfloat16)
    eb = np.ascontiguousarray(enc_b.reshape(NJ, 128).T)   # [128, NJ]
    db = np.ascontiguousarray(np.broadcast_to(dec_b, (128, D)))
    ones = np.ones((128, 128), np.float32)
    if DIST_MODE == "bf16r4":
        import ml_dtypes
        ones = np.asarray(ones, ml_dtypes.bfloat16)

    common = {"cbs": cbs, "cb": cb, "cbx": cbx, "ewT": ewT, "dwT": dwT,
              "eb": eb, "db": db, "ones": ones}
    if rescue:
        common["ebf"] = np.ascontiguousarray(np.broadcast_to(enc_b, (128, D)))

    in_maps = []
    for s in range(NCORES):
        shard = flat[s * TL:(s + 1) * TL]                 # [TL, D]
        xT = np.ascontiguousarray(
            shard.T.reshape(NJ, 128, TL).transpose(1, 0, 2))
        in_maps.append({"xT": xT, **common})
    return in_maps


def _run(inputs, trace=False):
    nc = _get_nc()
    in_maps = _host_prep(**inputs)
    res = run_bass_kernel_spmd(nc, in_maps, list(range(NCORES)), trace=trace)
    outs = [res.results[s]["out"] for s in range(NCORES)]
    full = np.concatenate(outs, axis=0).reshape(B, N, D)
    return full, res


def kernel(**inputs) -> np.ndarray:
    out, _ = _run(inputs, trace=False)
    return out


def kernel_traced(**inputs):
    out, res = _run(inputs, trace=True)
    return out, res

